# revision 27
# baseline (speedup 1.0000x reference)
"""Trainium2 Bass kernel: polar/cartesian ConvNext feature mix + 25-head scan.

Full (unsharded) inputs in, full output out. Internally: pure data-parallel
over the batch dim (32 -> 4 per core x 8 cores).

v7: software-pipelined ring loop; width-sum split over DVE/scalar/gpsimd.
  * All bulk inputs bf16 (f32 accumulation on device). One HWDGE stream on
    the sync queue in consumption order: smat, 8 cart tiles, 25 polar rings.
  * Per-ring polar width-sum (3072 elems) split: DVE pair-folds units 0-7
    (cc0-1) twice then reduces; gpsimd pair-folds units 8-9 (cc2,b0-1,
    host-packed contiguous); scalar's activation-accumulator sums units
    10-11. fe layout is flat [128, ring, 12] with natural (cc,b) order.
  * The per-position emission interleaves, at position P: folds(P),
    scan p(P-9), scan x(P-8) (p before x -- x(k) consumes acc from p(k-1)),
    PE head-matmul group (P-7), and the cart-phase DVE copies at positions
    2/4/6/8 -- so the in-order engine streams never serialize polar work
    behind the cart phase and PSUM head tiles stay within 3 buffers.
  * b1_eff enters via a K=1 PE matmul into the same PSUM accumulator.
  * grid_sample+mean == cart @ S (S built host-side from grid, bf16);
    /256 folded into W1; b2[r-1] recurrence folded into b1[r];
    gelu(exact) == 0.5*x*(1+tanh(c*x)) to <1e-7 for these head inputs.
"""
import numpy as np
import ml_dtypes

import concourse.bacc as bacc
import concourse.mybir as mybir
import concourse.tile as tile
from concourse import bass_utils
from concourse.masks import make_identity

F32 = mybir.dt.float32
BF16 = mybir.dt.bfloat16
AF = mybir.ActivationFunctionType
ALU = mybir.AluOpType
AX = mybir.AxisListType

# Problem shapes (fixed by the spec)
B, C, RHO, WP = 32, 384, 25, 256
HC = WC = 64
NPIX = HC * WC            # 4096
D = 2 * C                 # 768
NH = 40
NCORES = 8
BPC = B // NCORES         # 4
CCH = C // 128            # 3 channel chunks
KCH = NPIX // 128         # 32 pixel chunks
DCH = D // 128            # 6 feature chunks
KHALF = KCH // 2          # 16 pixel chunks per cart DMA
WH = WP // 2              # 128
LAG = 8                   # rings between fold and scan in the pipeline

GC = 0.7978845608028654   # sqrt(2/pi)

TRACE = False             # test harness may flip this for profiling
TRACE_KW: dict = {}
LAST_RESULTS = None


def _build_smat(grid):
    """[B, 4096, 25] f32: summed bilinear weights per (pixel, ring).

    Index math replicates the reference exactly (f32 floor/clip)."""
    gx = grid[..., 0].astype(np.float32)
    gy = grid[..., 1].astype(np.float32)
    ix = (gx + np.float32(1.0)) * np.float32(WC * 0.5) - np.float32(0.5)
    iy = (gy + np.float32(1.0)) * np.float32(HC * 0.5) - np.float32(0.5)
    ix0 = np.floor(ix)
    iy0 = np.floor(iy)
    tx = ix - ix0
    ty = iy - iy0
    corners = (
        (ix0, iy0, (1 - tx) * (1 - ty)),
        (ix0 + 1, iy0, tx * (1 - ty)),
        (ix0, iy0 + 1, (1 - tx) * ty),
        (ix0 + 1, iy0 + 1, tx * ty),
    )
    boff = np.arange(B, dtype=np.int64)[:, None, None] * (NPIX * RHO)
    roff = np.arange(RHO, dtype=np.int64)[None, :, None]
    keys = []
    vals = []
    for xi, yi, w in corners:
        valid = (xi >= 0) & (xi < WC) & (yi >= 0) & (yi < HC)
        xc = np.clip(xi, 0, WC - 1).astype(np.int64)
        yc = np.clip(yi, 0, HC - 1).astype(np.int64)
        keys.append((boff + (yc * WC + xc) * RHO + roff).ravel())
        vals.append((w * valid).astype(np.float64).ravel())
    s = np.bincount(np.concatenate(keys), weights=np.concatenate(vals),
                    minlength=B * NPIX * RHO)
    return s.reshape(B, NPIX, RHO).astype(np.float32)


def _build_program():
    nc = bacc.Bacc("TRN2", target_bir_lowering=False, debug=False,
                   enable_asserts=False, num_devices=NCORES)
    polar = nc.dram_tensor("polar", [RHO, 128, CCH * BPC * WP], BF16,
                           kind="ExternalInput")
    cart = nc.dram_tensor("cart", [BPC, 128, KCH, C], BF16,
                          kind="ExternalInput")
    smat = nc.dram_tensor("smat", [128, BPC, KCH, RHO], BF16,
                          kind="ExternalInput")
    w1 = nc.dram_tensor("w1", [128, RHO, DCH, NH], BF16, kind="ExternalInput")
    wrec = nc.dram_tensor("wrec", [BPC, RHO, NH], F32, kind="ExternalInput")
    b1r = nc.dram_tensor("b1r", [1, RHO, NH], BF16, kind="ExternalInput")
    w2h = nc.dram_tensor("w2h", [BPC, RHO, NH], F32, kind="ExternalInput")
    b2b = nc.dram_tensor("b2b", [BPC, RHO], F32, kind="ExternalInput")
    out = nc.dram_tensor("out", [BPC, RHO], F32, kind="ExternalOutput")

    with tile.TileContext(nc) as tc:
        with (
            tc.tile_pool(name="sing", bufs=1) as sing,
            tc.tile_pool(name="ppool", bufs=8) as ppool,
            tc.tile_pool(name="cpool", bufs=8) as cpool,
            tc.tile_pool(name="fcpool", bufs=2) as fcpool,
            tc.tile_pool(name="f1pool", bufs=3) as f1pool,
            tc.tile_pool(name="scanw", bufs=3) as scanw,
            tc.tile_pool(name="cps", bufs=2, space="PSUM") as cps,
            tc.tile_pool(name="tps", bufs=3, space="PSUM") as tps,
            tc.tile_pool(name="hps", bufs=3, space="PSUM") as hps,
        ):
            stile = sing.tile([128, BPC, KCH, RHO], BF16)
            w1_sb = sing.tile([128, RHO, DCH, NH], BF16)
            wrec_sb = sing.tile([BPC, RHO, NH], F32)
            b1r_sb = sing.tile([1, RHO, NH], BF16)
            ones4 = sing.tile([1, BPC], BF16)
            w2h_sb = sing.tile([BPC, RHO, NH], F32)
            b2b_sb = sing.tile([BPC, RHO], F32)
            ident = sing.tile([RHO, RHO], F32)
            fe_pf = sing.tile([128, RHO, 12], F32)
            fe_pb = sing.tile([128, RHO, 12], BF16)
            fe_cart = sing.tile([128, CCH, RHO, BPC], BF16)
            acc_sb = sing.tile([BPC, RHO], F32)

            # --- one HWDGE stream in consumption order: smat, cart, polar
            nc.sync.dma_start(out=stile, in_=smat.ap())
            ctls = []
            for b in range(BPC):
                for half in range(2):
                    ctl = cpool.tile([128, KHALF, C], BF16, tag="c",
                                     name=f"c{b}_{half}")
                    k0 = half * KHALF
                    nc.sync.dma_start(
                        out=ctl, in_=cart.ap()[b][:, k0:k0 + KHALF, :])
                    ctls.append(ctl)
            # pr half-row grid: rows 0-9 = low width-halves of units 0-9,
            # rows 10-19 = high halves, rows 20-23 = units 10-11 full
            prs = []
            for r in range(RHO):
                pr = ppool.tile([128, 24, WH], BF16, tag="p", name=f"p{r}")
                nc.sync.dma_start(out=pr, in_=polar.ap()[r])
                prs.append(pr)

            # --- consts via gpsimd (SWDGE), overlapping the stream ---
            make_identity(nc, ident)
            nc.gpsimd.memset(ones4, 1.0)
            nc.gpsimd.dma_start(out=w1_sb, in_=w1.ap())
            nc.gpsimd.dma_start(out=wrec_sb, in_=wrec.ap())
            nc.gpsimd.dma_start(out=b1r_sb, in_=b1r.ap())
            nc.gpsimd.dma_start(out=w2h_sb, in_=w2h.ap())
            nc.gpsimd.dma_start(out=b2b_sb, in_=b2b.ap())

            # --- cart matmuls + transposes on the PE (DVE copies are
            #     interleaved into the position loop below) ---
            cpsums, tpss = [], []
            for b in range(BPC):
                cpsum = cps.tile([RHO, C], F32, tag="cp", name=f"cp{b}")
                for k in range(KCH):
                    nc.tensor.matmul(
                        cpsum, stile[:, b, k, :],
                        ctls[2 * b + k // KHALF][:, k % KHALF, :],
                        start=(k == 0), stop=(k == KCH - 1))
                cpsums.append(cpsum)

            def cart_copies(b):
                fecart = fcpool.tile([RHO, C], F32, tag="fc", name=f"fc{b}")
                nc.scalar.copy(out=fecart, in_=cpsums[b])
                for cc in range(CCH):
                    tp = tps.tile([128, RHO], F32, tag="tp", name=f"tp{b}_{cc}")
                    nc.tensor.transpose(
                        tp, fecart[:, cc * 128:(cc + 1) * 128], ident)
                    nc.vector.tensor_copy(out=fe_cart[:, cc, :, b], in_=tp)

            # pipeline: folds(P), p(P-9), x(P-8), PE head group (P-7),
            # cart copies at P in {2, 4, 6, 8}
            scan_x = [None] * RHO
            scan_t = [None] * RHO
            scan_xw = [None] * RHO
            for P in range(RHO + LAG + 1):
                if P < RHO:
                    r = P
                    pr = prs[r]
                    # DVE: units 0-6: fold, fold, then reduce all of f2b
                    f1 = f1pool.tile([128, 7, WH], BF16, tag="f1",
                                     name=f"f1_{r}")
                    nc.vector.tensor_tensor(
                        out=f1, in0=pr[:, 0:7, :], in1=pr[:, 10:17, :],
                        op=ALU.add)
                    f2b = f1pool.tile([128, 10, WH // 2], BF16, tag="f2",
                                      name=f"f2_{r}")
                    nc.vector.tensor_tensor(
                        out=f2b[:, 0:7, :],
                        in0=f1[:, :, 0:WH // 2], in1=f1[:, :, WH // 2:WH],
                        op=ALU.add)
                    # gpsimd: units 7-9
                    g1 = f1pool.tile([128, 3, WH], BF16, tag="g1",
                                     name=f"g1_{r}")
                    nc.gpsimd.tensor_tensor(
                        out=g1, in0=pr[:, 7:10, :], in1=pr[:, 17:20, :],
                        op=ALU.add)
                    nc.gpsimd.tensor_tensor(
                        out=f2b[:, 7:10, :], in0=g1[:, :, 0:WH // 2],
                        in1=g1[:, :, WH // 2:WH], op=ALU.add)
                    nc.vector.reduce_sum(
                        out=fe_pf[:, r, 0:10], in_=f2b, axis=AX.X)
                    # scalar: units 10-11 via activation accumulator
                    for j, u in enumerate((10, 11)):
                        junk = scanw.tile([128, WP], BF16, tag="junk",
                                          name=f"junk{r}_{j}")
                        nc.scalar.activation(
                            out=junk, in_=pr[:, 20 + 2 * j:22 + 2 * j, :],
                            func=AF.Copy,
                            accum_out=fe_pf[:, r, u:u + 1])
                    nc.scalar.copy(out=fe_pb[:, r], in_=fe_pf[:, r])
                    if P in (2, 4, 6, 7):
                        cart_copies({2: 0, 4: 1, 6: 2, 7: 3}[P])

                # scan p for ring P-9 (before x(P-8): x(k) needs acc(k-1))
                q = P - LAG - 1
                if 0 <= q < RHO:
                    p = scanw.tile([BPC, NH], F32, tag="pr", name=f"pp{q}")
                    nc.vector.scalar_tensor_tensor(
                        out=p, in0=scan_t[q], scalar=1.0, in1=scan_xw[q],
                        op0=ALU.add, op1=ALU.mult,
                        accum_out=acc_sb[:, q:q + 1])

                # PE head-matmul group for ring P-7
                m = P - LAG + 1
                if 0 <= m < RHO:
                    hp = hps.tile([BPC, NH], F32, tag="hp", name=f"hp{m}")
                    nc.tensor.matmul(hp, ones4, b1r_sb[:, m, :],
                                     start=True, stop=False)
                    for cc in range(CCH):
                        nc.tensor.matmul(hp, fe_pb[:, m, cc * 4:cc * 4 + 4],
                                         w1_sb[:, m, cc, :],
                                         start=False, stop=False)
                    for cc in range(CCH):
                        nc.tensor.matmul(hp, fe_cart[:, cc, m, :],
                                         w1_sb[:, m, CCH + cc, :],
                                         start=False, stop=(cc == CCH - 1))
                    scan_x[m] = (hp, m)

                # scan x/tanh/xw for ring P-8
                k = P - LAG
                if 0 <= k < RHO:
                    hp, _ = scan_x[k]
                    if k == 0:
                        x = scanw.tile([BPC, NH], F32, tag="x", name="x0")
                        nc.vector.tensor_copy(out=x, in_=hp)
                    else:
                        x = scanw.tile([BPC, NH], F32, tag="x", name=f"x{k}")
                        nc.vector.scalar_tensor_tensor(
                            out=x, in0=wrec_sb[:, k, :],
                            scalar=acc_sb[:, k - 1:k],
                            in1=hp, op0=ALU.mult, op1=ALU.add)
                    t = scanw.tile([BPC, NH], F32, tag="t", name=f"t{k}")
                    nc.scalar.activation(out=t, in_=x, func=AF.Tanh, scale=GC)
                    xw = scanw.tile([BPC, NH], F32, tag="xw", name=f"xw{k}")
                    nc.gpsimd.tensor_tensor(out=xw, in0=x,
                                            in1=w2h_sb[:, k, :], op=ALU.mult)
                    scan_t[k] = t
                    scan_xw[k] = xw

            outv = sing.tile([BPC, RHO], F32)
            nc.vector.tensor_add(outv, acc_sb, b2b_sb)
            nc.vector.tensor_scalar(out=outv, in0=outv,
                                    scalar1=0.0, scalar2=float(np.pi),
                                    op0=ALU.max, op1=ALU.min)
            nc.gpsimd.dma_start(out=out.ap(), in_=outv)

    nc.finalize()
    return nc


def kernel(polar_feat, cart_feat, grid, W1_0, b1_0, W2_0, b2_0,
           W1s, b1s, W2s, b2s):
    global LAST_RESULTS
    f = np.float32
    bf = ml_dtypes.bfloat16
    polar_feat = np.ascontiguousarray(polar_feat, f)
    cart_feat = np.ascontiguousarray(cart_feat, f)
    grid = np.asarray(grid, f)

    smat = _build_smat(grid)                                   # [32, 4096, 25]
    polar_p = polar_feat.reshape(B, CCH, 128, RHO, WP).astype(bf)
    cart_p = cart_feat.reshape(B, C, KCH, 128).transpose(0, 3, 2, 1).astype(bf)
    smat_p = smat.reshape(B, KCH, 128, RHO).transpose(0, 2, 1, 3).astype(bf)

    W1c = np.concatenate([np.asarray(W1_0, f)[None],
                          np.asarray(W1s, f)[:, :D, :]], 0) / f(WP)
    w1_p = np.ascontiguousarray(
        W1c.reshape(RHO, DCH, 128, NH).transpose(2, 0, 1, 3)).astype(bf)
    wr = np.concatenate([np.zeros((1, NH), f), np.asarray(W1s, f)[:, D, :]], 0)
    b1 = np.concatenate([np.asarray(b1_0, f)[None], np.asarray(b1s, f)], 0)
    b2 = np.concatenate([np.asarray(b2_0, f)[None], np.asarray(b2s, f)], 0)[:, 0]
    W2 = np.concatenate([np.asarray(W2_0, f)[None], np.asarray(W2s, f)], 0)[:, :, 0]
    b1_eff = b1.copy()
    b1_eff[1:] += wr[1:] * b2[:-1, None]

    wrec_b = np.ascontiguousarray(np.broadcast_to(wr[None], (BPC, RHO, NH)))
    b1r_b = np.ascontiguousarray(b1_eff[None]).astype(bf)       # [1, 25, 40]
    w2h_b = np.ascontiguousarray(
        np.broadcast_to((W2 * f(0.5))[None], (BPC, RHO, NH)))
    b2b_b = np.ascontiguousarray(np.broadcast_to(b2[None], (BPC, RHO)))

    nc = _build_program()
    in_maps = []
    for core in range(NCORES):
        b0 = core * BPC
        # [bpc, cc, p, r, w] -> [r, p, unit(cc,b), w]; half-row grid:
        # rows 0-9 = low halves of units 0-9, 10-19 = high halves,
        # rows 20-23 = units 10-11 full width
        arr = polar_p[b0:b0 + BPC].transpose(3, 2, 1, 0, 4)   # [r,p,cc,b,w]
        arrU = arr.reshape(RHO, 128, 12, WP)
        lo10 = arrU[:, :, 0:10, 0:WH]
        hi10 = arrU[:, :, 0:10, WH:WP]
        tail = arrU[:, :, 10:12, :].reshape(RHO, 128, 4, WH)
        pol = np.ascontiguousarray(np.concatenate(
            [lo10, hi10, tail], axis=2)).reshape(RHO, 128, 24 * WH)
        in_maps.append({
            "polar": pol,
            "cart": np.ascontiguousarray(cart_p[b0:b0 + BPC]),
            "smat": np.ascontiguousarray(
                smat_p[b0:b0 + BPC].transpose(1, 0, 2, 3)),
            "w1": w1_p,
            "wrec": wrec_b,
            "b1r": b1r_b,
            "w2h": w2h_b,
            "b2b": b2b_b,
        })
    res = bass_utils.run_bass_kernel_spmd(
        nc, in_maps, core_ids=list(range(NCORES)), trace=TRACE, **TRACE_KW)
    LAST_RESULTS = res
    return np.concatenate([r["out"] for r in res.results], axis=0)
